# revision 7
# baseline (speedup 1.0000x reference)
"""Trainium2 Bass kernel for nn_CriterionLP (LP contrastive criterion loss).

Reference computation (B=2048 anchors, M=16384 supports, C=256, K=128 label
groups of G=128 supports each):
    sim   = (feats @ Fs.T) / TEMP                  [B, M]
    E     = exp(sim) grouped into K blocks of G    [B, K, G]
    pos   = exp(min sim over own-label block)      (one block per row)
    neg   = sum over other blocks of exp(max sim over block)
    loss  = mean_b( -log(pos/(pos+neg+eps) + eps) )

v15 design (support-sharded, 8 cores, no on-device collective):
  - fp8 e4m3 DoubleRow matmuls: full C=256 contraction per instruction,
    ~250ns effective per [128, 512] output (2x bf16), half the input DMA.
    fp8 end-to-end rel err ~6e-4 against a 2e-2 tolerance.
  - TRN2 engine rules pin the consumer design: gpsimd has no PSUM access
    and no max op; Act has no max; DVE may read only ONE operand from
    PSUM per op. All comparisons run on DVE, fed by Act copies:
      A2-bts: Act copies the two 64-col halves of each block separately
              (bf16 SBUF); DVE's first TT max is then SBUF x SBUF at 2x,
              and the rest of the tree is a 2x TT chain batched over 4
              b-tiles.
      H-bts:  Act copies only the upper halves; DVE's first TT maxes the
              PSUM half against the SBUF half (one PSUM operand, 1x),
              then a 2x chain over pairs.
    8 A2 + 8 H interleaved balances Act (~25us) and DVE (~25us).
  - min stats are needed only for own-label blocks; after the row
    rotation they are b-tile 0 groups 0..7 and b-tile 1 groups 8..15
    (both A2), covered by two small TT-min chains off the copies.
  - each core ships final [128, 16, 16] block maxes (+[128, 2, 8] mins);
    the HOST does exp, label masks, the cross-core sum and the -log mean
    (the gather/unshard step). No AllReduce, no cross-core barrier.
"""

import numpy as np
import ml_dtypes

import concourse.bass as bass
import concourse.bacc as bacc
import concourse.tile as tile
import concourse.mybir as mybir
from concourse.bass_utils import run_bass_kernel_spmd

VERSION_TAG = "v15"

F32 = mybir.dt.float32
BF16 = mybir.dt.bfloat16
F8 = mybir.dt.float8e4
AX = mybir.AxisListType
ALU = mybir.AluOpType
DRMODE = mybir.MatmulPerfMode.DoubleRow

TEMP = 0.05
EPS = 1e-6
SCALE = 16.0                # fp8 quantization scale (scores come out *S^2)
B, C = 2048, 256
NCORES = 8
KTOT, G = 128, 128          # label groups, supports per group
MLOC = 2048                 # support rows per core
KLOC = KTOT // NCORES       # groups per core (16)
NBT = B // 128              # b tiles of 128 rows (16)

# A2 batches of 4 consecutive bts; H pairs. bts 0,1 must be A2 (min path).
A2_BATCHES = [(0, 4), (8, 4)]
H_PAIRS = [(4, 2), (6, 2), (12, 2), (14, 2)]
# processing order interleaves A2 and H so Act and DVE both stay busy
BT_ORDER = [0, 4, 1, 5, 2, 6, 3, 7, 8, 12, 9, 13, 10, 14, 11, 15]

_PROG_CACHE = {}
LAST_RESULT = None          # BassKernelResults of the most recent run


def _route():
    r = {}
    for s, n in A2_BATCHES:
        for k in range(n):
            r[s + k] = ("A", s, n, k)
    for s, n in H_PAIRS:
        for k in range(n):
            r[s + k] = ("H", s, n, k)
    assert sorted(r) == list(range(NBT))
    return r


def _chain(nc, pool, src, out_ap, dims, w0, op, tag):
    """[128, *dims, w0] bf16 -> [128, *dims] via TT halving chain (2x)."""
    cur, w = src, w0
    while w > 1:
        hw = w // 2
        if hw == 1:
            nxt_ap = out_ap
        else:
            nxt = pool.tile([128, *dims, hw], BF16, name=f"c{hw}{tag}",
                            tag=f"c{hw}_{'_'.join(map(str, dims))}", bufs=2)
            nxt_ap = nxt[:]
        sel0 = (slice(None),) * (1 + len(dims)) + (slice(0, hw),)
        sel1 = (slice(None),) * (1 + len(dims)) + (slice(hw, w),)
        nc.vector.tensor_tensor(nxt_ap, cur[sel0], cur[sel1], op)
        if hw > 1:
            cur = nxt
        w = hw


def _build(fast):
    if fast in _PROG_CACHE:
        return _PROG_CACHE[fast]

    nc = bacc.Bacc("TRN2", target_bir_lowering=False, debug=False,
                   num_devices=NCORES)
    ft0d = nc.dram_tensor("ftq0", [128, 2, 1024], F8, kind="ExternalInput")
    ft1d = nc.dram_tensor("ftq1", [128, 2, 1024], F8, kind="ExternalInput")
    fs0d = nc.dram_tensor("fsq0", [128, 2, 1024], F8, kind="ExternalInput")
    fs1d = nc.dram_tensor("fsq1", [128, 2, 1024], F8, kind="ExternalInput")
    statd = nc.dram_tensor("stat", [128, NBT, KLOC], BF16,
                           kind="ExternalOutput")
    mind = nc.dram_tensor("mins", [128, 2, 8] if fast else [128, NBT, KLOC],
                          BF16, kind="ExternalOutput")

    route = _route()

    with tile.TileContext(nc) as tc:
        with (
            tc.tile_pool(name="wpool", bufs=1) as wp,
            tc.tile_pool(name="apool", bufs=2) as ap_,
            tc.tile_pool(name="hpool", bufs=3) as hp,
            tc.tile_pool(name="bpool", bufs=1) as bp,
            tc.tile_pool(name="tpool", bufs=2) as trp,
            tc.tile_pool(name="pspool", bufs=2, space="PSUM") as psp,
        ):
            # parallel DMA issue across engines; ft0+fs0+fs1 gate bt0
            ftt = [wp.tile([128, 2, 1024], F8, name=f"ft{c}") for c in range(2)]
            fst = [wp.tile([128, 2, 1024], F8, name=f"fs{c}") for c in range(2)]
            nc.sync.dma_start(ftt[0][:, :, :], ft0d[:, :, :])
            nc.scalar.dma_start(fst[0][:, :, :], fs0d[:, :, :])
            nc.gpsimd.dma_start(fst[1][:, :, :], fs1d[:, :, :])
            nc.sync.dma_start(ftt[1][:, :, :], ft1d[:, :, :])

            stat = bp.tile([128, NBT, KLOC], BF16, name="stat")
            minstat = bp.tile([128, 2, 8] if fast else [128, NBT, KLOC],
                              BF16, name="minstat")

            acp = {}     # batch start -> (A-half tile, B-half tile)
            l1o = {}     # batch start -> L1 output collection
            hx = {}
            hmn = {}
            mn_l1 = {}
            for bt in BT_ORDER:
                kind, s, n, k = route[bt]
                ps = psp.tile([128, 2048], F32, name=f"ps{bt}", tag="ps")
                ftc = ftt[bt // 8]
                bl = (bt % 8) * 128
                for m in range(4):
                    nc.tensor.matmul(
                        ps[:, m * 512:(m + 1) * 512],
                        ftc[:, :, bl:bl + 128],
                        fst[m // 2][:, :, (m % 2) * 512:(m % 2) * 512 + 512],
                        start=True, stop=True, perf_mode=DRMODE,
                    )
                ps3 = ps.rearrange("p (k g) -> p k g", g=128)
                if kind == "A":
                    if k == 0:
                        acp[s] = (ap_.tile([128, n, KLOC, 64], BF16,
                                           name=f"aca{s}", tag="aca"),
                                  ap_.tile([128, n, KLOC, 64], BF16,
                                           name=f"acb{s}", tag="acb"))
                        l1o[s] = ap_.tile([128, n, KLOC, 64], BF16,
                                          name=f"l1o{s}", tag="l1o")
                    ca, cb = acp[s]
                    nc.scalar.copy(ca[:, k, :, :], ps3[:, :, 0:64])
                    nc.scalar.copy(cb[:, k, :, :], ps3[:, :, 64:128])
                    nc.vector.tensor_tensor(l1o[s][:, k, :, :],
                                            ca[:, k, :, :], cb[:, k, :, :],
                                            ALU.max)
                    if fast and bt in (0, 1):
                        gs = slice(bt * 8, bt * 8 + 8)
                        mt = trp.tile([128, 8, 64], BF16, name=f"mnl{bt}",
                                      tag="mnl", bufs=2)
                        nc.vector.tensor_tensor(mt[:], ca[:, k, gs, :],
                                                cb[:, k, gs, :], ALU.min)
                        _chain(nc, trp, mt, minstat[:, bt, :], [8], 64,
                               ALU.min, f"mn{bt}")
                    elif not fast:
                        if k == 0:
                            mn_l1[s] = ap_.tile([128, n, KLOC, 64], BF16,
                                                name=f"mnc{s}", tag="mnc")
                        nc.vector.tensor_tensor(mn_l1[s][:, k, :, :],
                                                ca[:, k, :, :],
                                                cb[:, k, :, :], ALU.min)
                        if k == n - 1:
                            _chain(nc, trp, mn_l1[s],
                                   minstat[:, s:s + n, :], [n, KLOC], 64,
                                   ALU.min, f"amn{s}")
                    if k == n - 1:
                        _chain(nc, trp, l1o[s], stat[:, s:s + n, :],
                               [n, KLOC], 64, ALU.max, f"ax{s}")
                else:
                    hpb = hp.tile([128, KLOC, 64], BF16, name=f"hp{bt}",
                                  tag="hpb")
                    nc.scalar.copy(hpb[:, :, :], ps3[:, :, 64:128])
                    if k == 0:
                        hx[s] = ap_.tile([128, n, KLOC, 64], BF16,
                                         name=f"hx{s}", tag="hx")
                        if not fast:
                            hmn[s] = ap_.tile([128, n, KLOC, 64], BF16,
                                              name=f"hmn{s}", tag="hmn")
                    nc.vector.tensor_tensor(hx[s][:, k, :, :],
                                            ps3[:, :, 0:64], hpb[:, :, :],
                                            ALU.max)
                    if not fast:
                        nc.vector.tensor_tensor(hmn[s][:, k, :, :],
                                                ps3[:, :, 0:64],
                                                hpb[:, :, :], ALU.min)
                    if k == n - 1:
                        _chain(nc, trp, hx[s], stat[:, s:s + n, :],
                               [n, KLOC], 64, ALU.max, f"hx{s}")
                        if not fast:
                            _chain(nc, trp, hmn[s],
                                   minstat[:, s:s + n, :], [n, KLOC], 64,
                                   ALU.min, f"hn{s}")

            nc.sync.dma_start(statd[:, :, :], stat[:, :, :])
            nc.sync.dma_start(mind[:, :] if fast else mind[:, :, :],
                              minstat[:, :] if fast else minstat[:, :, :])

    nc.compile()
    _PROG_CACHE[fast] = nc
    return nc


def _quant(x):
    return np.clip(x * SCALE, -240.0, 240.0).astype(ml_dtypes.float8_e4m3fn)


def kernel(feats, feats_s, labels, labels_s, topk, num_instances):
    global LAST_RESULT
    feats = np.asarray(feats, dtype=np.float32)
    feats_s = np.asarray(feats_s, dtype=np.float32)
    labels = np.asarray(labels).astype(np.int64).ravel()
    labels_s = np.asarray(labels_s).astype(np.int64).ravel()
    tk, ni = int(topk), int(num_instances)
    assert feats.shape == (B, C), feats.shape
    assert tk * ni == G and feats_s.shape == (B, tk, C)

    Fs = feats_s.reshape(-1, C)                       # [16384, 256]
    glab = labels_s.reshape(KTOT, G)[:, 0]            # label of each block

    # fast path valid if each core's own-label rows are exactly the
    # contiguous global rows [256j, 256j+256) (reference's structured labels)
    fast = bool(np.array_equal(labels_s, np.repeat(labels, tk)))
    if fast:
        for j in range(NCORES):
            own = np.isin(labels, glab[j * KLOC:(j + 1) * KLOC])
            want = np.zeros(B, dtype=bool)
            want[j * (B // NCORES):(j + 1) * (B // NCORES)] = True
            if not np.array_equal(own, want):
                fast = False
                break

    nc = _build(fast)

    in_maps = []
    for j in range(NCORES):
        shift = (B // NCORES) * j
        f_loc = np.roll(feats, -shift, axis=0) if fast else feats
        # lhsT layout [kp, kt, b]: feats_loc.T is [c, b] = [kt*128+kp, b]
        ftT = np.ascontiguousarray(
            f_loc.T.reshape(2, 128, B).transpose(1, 0, 2))
        fsT = Fs[j * MLOC:(j + 1) * MLOC].T.reshape(2, 128, MLOC)
        fsT = np.ascontiguousarray(fsT.transpose(1, 0, 2))   # [kp, kt, n]
        in_maps.append({
            "ftq0": _quant(ftT[:, :, 0:1024]),
            "ftq1": _quant(ftT[:, :, 1024:2048]),
            "fsq0": _quant(fsT[:, :, 0:1024]),
            "fsq1": _quant(fsT[:, :, 1024:2048]),
        })

    LAST_RESULT = run_bass_kernel_spmd(nc, in_maps, core_ids=list(range(NCORES)))

    # ---- host gather/unshard: exp, masks, cross-core sum, -log mean ----
    inv = 1.0 / (TEMP * SCALE * SCALE)
    pos = np.zeros(B, dtype=np.float64)
    neg = np.zeros(B, dtype=np.float64)
    for j in range(NCORES):
        res = LAST_RESULT.results[j]
        gl_j = glab[j * KLOC:(j + 1) * KLOC]              # [16]
        s = np.asarray(res["stat"], dtype=np.float32)     # [128, 16, 16]
        emax = np.exp(s.transpose(1, 0, 2).reshape(B, KLOC) * inv)
        lab_loc = np.roll(labels, -(B // NCORES) * j) if fast else labels
        gmask = lab_loc[:, None] == gl_j[None, :]         # [2048, 16]
        negj = np.where(gmask, 0.0, emax).sum(axis=1)
        mn = np.asarray(res["mins"], dtype=np.float32)
        posj = np.zeros(B, dtype=np.float64)
        if fast:
            emin = np.exp(mn * inv)                       # [128, 2, 8]
            for t in range(2):
                rows = slice(t * 128, (t + 1) * 128)
                gm = gmask[rows, t * 8:(t + 1) * 8]       # [128, 8]
                posj[rows] = np.where(gm, emin[:, t, :], 0.0).sum(axis=1)
        else:
            emin = np.exp(mn.transpose(1, 0, 2).reshape(B, KLOC) * inv)
            posj = np.where(gmask, emin, 0.0).sum(axis=1)
        if fast:
            shift = (B // NCORES) * j
            negj = np.roll(negj, shift)
            posj = np.roll(posj, shift)
        pos += posj
        neg += negj
    loss_i = -np.log(pos / (pos + neg + EPS) + EPS)
    return np.float32(loss_i.mean())


# revision 8
# speedup vs baseline: 1.1265x; 1.1265x over previous
"""Trainium2 Bass kernel for nn_CriterionLP (LP contrastive criterion loss).

Reference computation (B=2048 anchors, M=16384 supports, C=256, K=128 label
groups of G=128 supports each):
    sim   = (feats @ Fs.T) / TEMP                  [B, M]
    E     = exp(sim) grouped into K blocks of G    [B, K, G]
    pos   = exp(min sim over own-label block)      (one block per row)
    neg   = sum over other blocks of exp(max sim over block)
    loss  = mean_b( -log(pos/(pos+neg+eps) + eps) )

v16 design (support-sharded, 8 cores, no on-device collective):
  - fp8 e4m3 DoubleRow matmuls: full C=256 contraction per instruction,
    ~250ns effective per [128, 512] output; half the input DMA of bf16.
    fp8 end-to-end rel err ~6e-4 against a 2e-2 tolerance.
  - work unit = half b-tile [128, 1024] PSUM (2 banks, 4-deep ring) so
    the PE is never gated by a slow consumer holding a big tile.
  - TRN2 engine rules: gpsimd has no PSUM access and no max; Act has no
    max; DVE reads at most ONE PSUM operand per op. Split:
      D-units (8, b-tiles 0-3): one DVE tensor_reduce straight off PSUM
              -> final [128, 8] block maxes (W=1).
      A2-units (24, b-tiles 4-15): Act copies the two 64-col halves of
              each block separately to bf16 SBUF; DVE TT-maxes them at
              2x, then short 2x chains batched per (h, bt-range) cell
              down to W=16; the HOST finishes max-of-16.
    This balances Act (~29us) and DVE (~29us); PE (~16us) hides under.
  - min stats are needed only for own-label blocks; after the row
    rotation they live in D-units (0,h0) and (1,h1): two extra DVE
    min-reduces off the same PSUM.
  - outputs: stat [128,4,16] (W=1 rows 0-3), statw [128,12,16,16] (W=16
    rows 4-15, DMA'd in two h-chunks as cells finish), mins [128,2,8].
    The HOST does exp, label masks, cross-core sum, -log mean (the
    gather/unshard step). No AllReduce, no cross-core barrier.
"""

import numpy as np
import ml_dtypes

import concourse.bass as bass
import concourse.bacc as bacc
import concourse.tile as tile
import concourse.mybir as mybir
from concourse.bass_utils import run_bass_kernel_spmd

VERSION_TAG = "v16"

F32 = mybir.dt.float32
BF16 = mybir.dt.bfloat16
F8 = mybir.dt.float8e4
AX = mybir.AxisListType
ALU = mybir.AluOpType
DRMODE = mybir.MatmulPerfMode.DoubleRow

TEMP = 0.05
EPS = 1e-6
SCALE = 16.0                # fp8 quantization scale (scores come out *S^2)
B, C = 2048, 256
NCORES = 8
KTOT, G = 128, 128          # label groups, supports per group
MLOC = 2048                 # support rows per core
KLOC = KTOT // NCORES       # groups per core (16)
NBT = B // 128              # b tiles of 128 rows (16)
W = 16                      # shipped stat width for A2 rows
NA2B = NBT - 4              # A2 b-tiles (4..15)

# A2 chain cells: (h, first bt, n bts). 8-cells amortize chain overhead.
CELLS = [(0, 4, 8), (0, 12, 4), (1, 4, 8), (1, 12, 4)]

_PROG_CACHE = {}
LAST_RESULT = None          # BassKernelResults of the most recent run


def _units():
    d = [(0, 0), (1, 1), (0, 1), (1, 0), (2, 0), (2, 1), (3, 0), (3, 1)]
    a2 = [(bt, 0) for bt in range(4, 16)] + [(bt, 1) for bt in range(4, 16)]
    order = []
    di, ai = 0, 0
    for i in range(32):
        if i % 4 == 0 and di < len(d):
            order.append(("D", d[di])); di += 1
        else:
            order.append(("A", a2[ai])); ai += 1
    while ai < len(a2):
        order.append(("A", a2[ai])); ai += 1
    return order


def _chain(nc, pool, src, out_ap, dims, w0, w1, op, tag):
    """[128, *dims, w0] bf16 -> [128, *dims, w1] via TT halving (2x)."""
    cur, w = src, w0
    while w > w1:
        hw = w // 2
        if hw == w1:
            nxt_ap = out_ap
        else:
            nxt = pool.tile([128, *dims, hw], BF16, name=f"c{hw}{tag}",
                            tag=f"c{hw}_{'_'.join(map(str, dims))}", bufs=2)
            nxt_ap = nxt[:]
        sel0 = (slice(None),) * (1 + len(dims)) + (slice(0, hw),)
        sel1 = (slice(None),) * (1 + len(dims)) + (slice(hw, w),)
        nc.vector.tensor_tensor(nxt_ap, cur[sel0], cur[sel1], op)
        if hw > w1:
            cur = nxt
        w = hw


def _build(fast):
    if fast in _PROG_CACHE:
        return _PROG_CACHE[fast]

    nc = bacc.Bacc("TRN2", target_bir_lowering=False, debug=False,
                   num_devices=NCORES)
    ft0d = nc.dram_tensor("ftq0", [128, 2, 1024], F8, kind="ExternalInput")
    ft1d = nc.dram_tensor("ftq1", [128, 2, 1024], F8, kind="ExternalInput")
    fs0d = nc.dram_tensor("fsq0", [128, 2, 1024], F8, kind="ExternalInput")
    fs1d = nc.dram_tensor("fsq1", [128, 2, 1024], F8, kind="ExternalInput")
    statd = nc.dram_tensor("stat", [128, 4, KLOC], BF16,
                           kind="ExternalOutput")
    statwd = nc.dram_tensor("statw", [128, NA2B, KLOC, W], BF16,
                            kind="ExternalOutput")
    mind = nc.dram_tensor("mins", [128, 2, 8] if fast else [128, 4, KLOC],
                          BF16, kind="ExternalOutput")
    if not fast:
        minwd = nc.dram_tensor("minw", [128, NA2B, KLOC, W], BF16,
                               kind="ExternalOutput")

    units = _units()

    with tile.TileContext(nc) as tc:
        with (
            tc.tile_pool(name="wpool", bufs=1) as wp,
            tc.tile_pool(name="cpool", bufs=4) as cpp,
            tc.tile_pool(name="bpool", bufs=1) as bp,
            tc.tile_pool(name="tpool", bufs=2) as trp,
            tc.tile_pool(name="pspool", bufs=4, space="PSUM") as psp,
        ):
            ftt = [wp.tile([128, 2, 1024], F8, name=f"ft{c}") for c in range(2)]
            fst = [wp.tile([128, 2, 1024], F8, name=f"fs{c}") for c in range(2)]
            nc.sync.dma_start(ftt[0][:, :, :], ft0d[:, :, :])
            nc.scalar.dma_start(fst[0][:, :, :], fs0d[:, :, :])
            nc.gpsimd.dma_start(fst[1][:, :, :], fs1d[:, :, :])
            nc.sync.dma_start(ftt[1][:, :, :], ft1d[:, :, :])

            stat = bp.tile([128, 4, KLOC], BF16, name="stat")
            statw = bp.tile([128, NA2B, KLOC, W], BF16, name="statw")
            minstat = bp.tile([128, 2, 8] if fast else [128, 4, KLOC],
                              BF16, name="minstat")
            if not fast:
                minw = bp.tile([128, NA2B, KLOC, W], BF16, name="minw")

            # L1-output collection tiles per cell
            l1c = {(h, s): bp.tile([128, n, 8, 64], BF16, name=f"l1c{h}{s}")
                   for (h, s, n) in CELLS}
            if not fast:
                l1m = {(h, s): bp.tile([128, n, 8, 64], BF16,
                                       name=f"l1m{h}{s}")
                       for (h, s, n) in CELLS}
            cells_done = set()

            for kind, (bt, h) in units:
                ps = psp.tile([128, 1024], F32, name=f"ps{bt}_{h}", tag="ps")
                ftc = ftt[bt // 8]
                bl = (bt % 8) * 128
                for m in range(2):
                    nc.tensor.matmul(
                        ps[:, m * 512:(m + 1) * 512],
                        ftc[:, :, bl:bl + 128],
                        fst[h][:, :, m * 512:(m + 1) * 512],
                        start=True, stop=True, perf_mode=DRMODE,
                    )
                ps3 = ps.rearrange("p (k g) -> p k g", g=128)
                ksl = slice(h * 8, (h + 1) * 8)
                if kind == "D":
                    nc.vector.tensor_reduce(stat[:, bt, ksl], ps3[:],
                                            axis=AX.X, op=ALU.max)
                    if fast:
                        if (bt, h) in ((0, 0), (1, 1)):
                            nc.vector.tensor_reduce(minstat[:, bt, :],
                                                    ps3[:], axis=AX.X,
                                                    op=ALU.min)
                    else:
                        nc.vector.tensor_reduce(minstat[:, bt, ksl], ps3[:],
                                                axis=AX.X, op=ALU.min)
                else:
                    ca = cpp.tile([128, 8, 64], BF16, name=f"ca{bt}_{h}",
                                  tag="ca")
                    cb = cpp.tile([128, 8, 64], BF16, name=f"cb{bt}_{h}",
                                  tag="cb")
                    nc.scalar.copy(ca[:, :, :], ps3[:, :, 0:64])
                    nc.scalar.copy(cb[:, :, :], ps3[:, :, 64:128])
                    # cell membership
                    for (ch, cs, cn) in CELLS:
                        if ch == h and cs <= bt < cs + cn:
                            break
                    nc.vector.tensor_tensor(l1c[(ch, cs)][:, bt - cs, :, :],
                                            ca[:], cb[:], ALU.max)
                    if not fast:
                        nc.vector.tensor_tensor(
                            l1m[(ch, cs)][:, bt - cs, :, :], ca[:], cb[:],
                            ALU.min)
                    if bt == cs + cn - 1:
                        cells_done.add((ch, cs))
                        wsl = slice(cs - 4, cs - 4 + cn)
                        _chain(nc, trp, l1c[(ch, cs)],
                               statw[:, wsl, ksl, :], [cn, 8], 64, W,
                               ALU.max, f"x{ch}{cs}")
                        if not fast:
                            _chain(nc, trp, l1m[(ch, cs)],
                                   minw[:, wsl, ksl, :], [cn, 8], 64, W,
                                   ALU.min, f"n{ch}{cs}")
                        if all((hh, ss) in cells_done
                               for (hh, ss, _) in CELLS if hh == ch):
                            # whole h-half of statw finished -> stream out
                            nc.sync.dma_start(
                                statwd[:, :, ksl, :], statw[:, :, ksl, :])
                            if not fast:
                                nc.scalar.dma_start(
                                    minwd[:, :, ksl, :], minw[:, :, ksl, :])

            nc.sync.dma_start(statd[:, :, :], stat[:, :, :])
            nc.sync.dma_start(mind[:, :] if fast else mind[:, :, :],
                              minstat[:, :] if fast else minstat[:, :, :])

    nc.compile()
    _PROG_CACHE[fast] = nc
    return nc


def _quant(x):
    return np.clip(x * SCALE, -240.0, 240.0).astype(ml_dtypes.float8_e4m3fn)


def kernel(feats, feats_s, labels, labels_s, topk, num_instances):
    global LAST_RESULT
    feats = np.asarray(feats, dtype=np.float32)
    feats_s = np.asarray(feats_s, dtype=np.float32)
    labels = np.asarray(labels).astype(np.int64).ravel()
    labels_s = np.asarray(labels_s).astype(np.int64).ravel()
    tk, ni = int(topk), int(num_instances)
    assert feats.shape == (B, C), feats.shape
    assert tk * ni == G and feats_s.shape == (B, tk, C)

    Fs = feats_s.reshape(-1, C)                       # [16384, 256]
    glab = labels_s.reshape(KTOT, G)[:, 0]            # label of each block

    fast = bool(np.array_equal(labels_s, np.repeat(labels, tk)))
    if fast:
        for j in range(NCORES):
            own = np.isin(labels, glab[j * KLOC:(j + 1) * KLOC])
            want = np.zeros(B, dtype=bool)
            want[j * (B // NCORES):(j + 1) * (B // NCORES)] = True
            if not np.array_equal(own, want):
                fast = False
                break

    nc = _build(fast)

    in_maps = []
    for j in range(NCORES):
        shift = (B // NCORES) * j
        f_loc = np.roll(feats, -shift, axis=0) if fast else feats
        ftT = np.ascontiguousarray(
            f_loc.T.reshape(2, 128, B).transpose(1, 0, 2))
        fsT = Fs[j * MLOC:(j + 1) * MLOC].T.reshape(2, 128, MLOC)
        fsT = np.ascontiguousarray(fsT.transpose(1, 0, 2))   # [kp, kt, n]
        in_maps.append({
            "ftq0": _quant(ftT[:, :, 0:1024]),
            "ftq1": _quant(ftT[:, :, 1024:2048]),
            "fsq0": _quant(fsT[:, :, 0:1024]),
            "fsq1": _quant(fsT[:, :, 1024:2048]),
        })

    LAST_RESULT = run_bass_kernel_spmd(nc, in_maps, core_ids=list(range(NCORES)))

    # ---- host gather/unshard: exp, masks, cross-core sum, -log mean ----
    inv = 1.0 / (TEMP * SCALE * SCALE)
    pos = np.zeros(B, dtype=np.float64)
    neg = np.zeros(B, dtype=np.float64)
    for j in range(NCORES):
        res = LAST_RESULT.results[j]
        gl_j = glab[j * KLOC:(j + 1) * KLOC]              # [16]
        s03 = np.asarray(res["stat"], dtype=np.float32)   # [128, 4, 16]
        sw = np.asarray(res["statw"], dtype=np.float32).max(axis=-1)
        s = np.concatenate([s03, sw], axis=1)             # [128, 16, 16]
        emax = np.exp(s.transpose(1, 0, 2).reshape(B, KLOC) * inv)
        lab_loc = np.roll(labels, -(B // NCORES) * j) if fast else labels
        gmask = lab_loc[:, None] == gl_j[None, :]         # [2048, 16]
        negj = np.where(gmask, 0.0, emax).sum(axis=1)
        mn = np.asarray(res["mins"], dtype=np.float32)
        posj = np.zeros(B, dtype=np.float64)
        if fast:
            emin = np.exp(mn * inv)                       # [128, 2, 8]
            for t in range(2):
                rows = slice(t * 128, (t + 1) * 128)
                gm = gmask[rows, t * 8:(t + 1) * 8]       # [128, 8]
                posj[rows] = np.where(gm, emin[:, t, :], 0.0).sum(axis=1)
        else:
            mw = np.asarray(res["minw"], dtype=np.float32).min(axis=-1)
            m_all = np.concatenate([mn, mw], axis=1)      # [128, 16, 16]
            emin = np.exp(m_all.transpose(1, 0, 2).reshape(B, KLOC) * inv)
            posj = np.where(gmask, emin, 0.0).sum(axis=1)
        if fast:
            shift = (B // NCORES) * j
            negj = np.roll(negj, shift)
            posj = np.roll(posj, shift)
        pos += posj
        neg += negj
    loss_i = -np.log(pos / (pos + neg + EPS) + EPS)
    return np.float32(loss_i.mean())


# revision 10
# speedup vs baseline: 1.2867x; 1.1421x over previous
"""Trainium2 Bass kernel for nn_CriterionLP (LP contrastive criterion loss).

Reference computation (B=2048 anchors, M=16384 supports, C=256, K=128 label
groups of G=128 supports each):
    sim   = (feats @ Fs.T) / TEMP                  [B, M]
    E     = exp(sim) grouped into K blocks of G    [B, K, G]
    pos   = exp(min sim over own-label block)      (one block per row)
    neg   = sum over other blocks of exp(max sim over block)
    loss  = mean_b( -log(pos/(pos+neg+eps) + eps) )

v16 design (support-sharded, 8 cores, no on-device collective):
  - fp8 e4m3 DoubleRow matmuls: full C=256 contraction per instruction,
    ~250ns effective per [128, 512] output; half the input DMA of bf16.
    fp8 end-to-end rel err ~6e-4 against a 2e-2 tolerance.
  - work unit = half b-tile [128, 1024] PSUM (2 banks, 4-deep ring) so
    the PE is never gated by a slow consumer holding a big tile.
  - TRN2 engine rules: gpsimd has no PSUM access and no max; Act has no
    max; DVE reads at most ONE PSUM operand per op. Split:
      D-units (8, b-tiles 0-3): one DVE tensor_reduce straight off PSUM
              -> final [128, 8] block maxes (W=1).
      A2-units (24, b-tiles 4-15): Act copies the two 64-col halves of
              each block separately to bf16 SBUF; DVE TT-maxes them at
              2x, then short 2x chains batched per (h, bt-range) cell
              down to W=16; the HOST finishes max-of-16.
    This balances Act (~29us) and DVE (~29us); PE (~16us) hides under.
  - min stats are needed only for own-label blocks; after the row
    rotation they live in D-units (0,h0) and (1,h1): two extra DVE
    min-reduces off the same PSUM.
  - outputs: stat [128,4,16] (W=1 rows 0-3), statw [128,12,16,16] (W=16
    rows 4-15, DMA'd in two h-chunks as cells finish), mins [128,2,8].
    The HOST does exp, label masks, cross-core sum, -log mean (the
    gather/unshard step). No AllReduce, no cross-core barrier.
"""

import numpy as np
import ml_dtypes

import concourse.bass as bass
import concourse.bacc as bacc
import concourse.tile as tile
import concourse.mybir as mybir
from concourse.bass_utils import run_bass_kernel_spmd

VERSION_TAG = "v16"

F32 = mybir.dt.float32
BF16 = mybir.dt.bfloat16
F8 = mybir.dt.float8e4
AX = mybir.AxisListType
ALU = mybir.AluOpType
DRMODE = mybir.MatmulPerfMode.DoubleRow

TEMP = 0.05
EPS = 1e-6
SCALE = 16.0                # fp8 quantization scale (scores come out *S^2)
B, C = 2048, 256
NCORES = 8
KTOT, G = 128, 128          # label groups, supports per group
MLOC = 2048                 # support rows per core
KLOC = KTOT // NCORES       # groups per core (16)
NBT = B // 128              # b tiles of 128 rows (16)
W = 16                      # shipped stat width for A2 rows
NA2B = NBT - 4              # A2 b-tiles (4..15)

# A2 chain cells: (h, first bt, n bts). 8-cells amortize chain overhead.
CELLS = [(0, 4, 8), (0, 12, 4), (1, 4, 8), (1, 12, 4)]

_PROG_CACHE = {}
LAST_RESULT = None          # BassKernelResults of the most recent run


def _units():
    d = [(0, 0), (1, 1), (0, 1), (1, 0), (2, 0), (2, 1), (3, 0), (3, 1)]
    a2 = [(bt, 0) for bt in range(4, 16)] + [(bt, 1) for bt in range(4, 16)]
    order = []
    di, ai = 0, 0
    for i in range(32):
        if i % 4 == 0 and di < len(d):
            order.append(("D", d[di])); di += 1
        else:
            order.append(("A", a2[ai])); ai += 1
    while ai < len(a2):
        order.append(("A", a2[ai])); ai += 1
    return order


def _chain(nc, pool, src, out_ap, dims, w0, w1, op, tag):
    """[128, *dims, w0] bf16 -> [128, *dims, w1] via TT halving (2x)."""
    cur, w = src, w0
    while w > w1:
        hw = w // 2
        if hw == w1:
            nxt_ap = out_ap
        else:
            nxt = pool.tile([128, *dims, hw], BF16, name=f"c{hw}{tag}",
                            tag=f"c{hw}_{'_'.join(map(str, dims))}", bufs=2)
            nxt_ap = nxt[:]
        sel0 = (slice(None),) * (1 + len(dims)) + (slice(0, hw),)
        sel1 = (slice(None),) * (1 + len(dims)) + (slice(hw, w),)
        nc.vector.tensor_tensor(nxt_ap, cur[sel0], cur[sel1], op)
        if hw > w1:
            cur = nxt
        w = hw


def _build(fast):
    if fast in _PROG_CACHE:
        return _PROG_CACHE[fast]

    nc = bacc.Bacc("TRN2", target_bir_lowering=False, debug=False,
                   num_devices=NCORES)
    ft0d = nc.dram_tensor("ftq0", [128, 2, 1024], F8, kind="ExternalInput")
    ft1d = nc.dram_tensor("ftq1", [128, 2, 1024], F8, kind="ExternalInput")
    fs0d = nc.dram_tensor("fsq0", [128, 2, 1024], F8, kind="ExternalInput")
    fs1d = nc.dram_tensor("fsq1", [128, 2, 1024], F8, kind="ExternalInput")
    statd = nc.dram_tensor("stat", [128, 4, KLOC], BF16,
                           kind="ExternalOutput")
    statwd = nc.dram_tensor("statw", [128, NA2B, KLOC, W], BF16,
                            kind="ExternalOutput")
    mind = nc.dram_tensor("mins", [128, 2, 8] if fast else [128, 4, KLOC],
                          BF16, kind="ExternalOutput")
    if not fast:
        minwd = nc.dram_tensor("minw", [128, NA2B, KLOC, W], BF16,
                               kind="ExternalOutput")

    units = _units()

    with tile.TileContext(nc) as tc:
        with (
            tc.tile_pool(name="wpool", bufs=1) as wp,
            tc.tile_pool(name="cpool", bufs=4) as cpp,
            tc.tile_pool(name="bpool", bufs=1) as bp,
            tc.tile_pool(name="tpool", bufs=2) as trp,
            tc.tile_pool(name="pspool", bufs=4, space="PSUM") as psp,
        ):
            ftt = [wp.tile([128, 2, 1024], F8, name=f"ft{c}") for c in range(2)]
            fst = [wp.tile([128, 2, 1024], F8, name=f"fs{c}") for c in range(2)]
            nc.sync.dma_start(ftt[0][:, :, :], ft0d[:, :, :])
            nc.scalar.dma_start(fst[0][:, :, :], fs0d[:, :, :])
            nc.sync.dma_start(fst[1][:, :, :], fs1d[:, :, :])
            nc.sync.dma_start(ftt[1][:, :, :], ft1d[:, :, :])

            stat = bp.tile([128, 4, KLOC], BF16, name="stat")
            statw = bp.tile([128, NA2B, KLOC, W], BF16, name="statw")
            minstat = bp.tile([128, 2, 8] if fast else [128, 4, KLOC],
                              BF16, name="minstat")
            if not fast:
                minw = bp.tile([128, NA2B, KLOC, W], BF16, name="minw")

            # L1-output collection tiles per cell
            l1c = {(h, s): bp.tile([128, n, 8, 64], BF16, name=f"l1c{h}{s}")
                   for (h, s, n) in CELLS}
            if not fast:
                l1m = {(h, s): bp.tile([128, n, 8, 64], BF16,
                                       name=f"l1m{h}{s}")
                       for (h, s, n) in CELLS}
            cells_done = set()

            for kind, (bt, h) in units:
                ps = psp.tile([128, 1024], F32, name=f"ps{bt}_{h}", tag="ps")
                ftc = ftt[bt // 8]
                bl = (bt % 8) * 128
                for m in range(2):
                    nc.tensor.matmul(
                        ps[:, m * 512:(m + 1) * 512],
                        ftc[:, :, bl:bl + 128],
                        fst[h][:, :, m * 512:(m + 1) * 512],
                        start=True, stop=True, perf_mode=DRMODE,
                    )
                ps3 = ps.rearrange("p (k g) -> p k g", g=128)
                ksl = slice(h * 8, (h + 1) * 8)
                if kind == "D":
                    nc.vector.tensor_reduce(stat[:, bt, ksl], ps3[:],
                                            axis=AX.X, op=ALU.max)
                    if fast:
                        if (bt, h) in ((0, 0), (1, 1)):
                            nc.vector.tensor_reduce(minstat[:, bt, :],
                                                    ps3[:], axis=AX.X,
                                                    op=ALU.min)
                    else:
                        nc.vector.tensor_reduce(minstat[:, bt, ksl], ps3[:],
                                                axis=AX.X, op=ALU.min)
                else:
                    cp = cpp.tile([128, 8, 128], BF16, name=f"cp{bt}_{h}",
                                  tag="cp")
                    nc.scalar.copy(cp[:, :, :], ps[:, :])
                    # cell membership
                    for (ch, cs, cn) in CELLS:
                        if ch == h and cs <= bt < cs + cn:
                            break
                    nc.vector.tensor_tensor(l1c[(ch, cs)][:, bt - cs, :, :],
                                            cp[:, :, 0:64], cp[:, :, 64:128],
                                            ALU.max)
                    if not fast:
                        nc.vector.tensor_tensor(
                            l1m[(ch, cs)][:, bt - cs, :, :],
                            cp[:, :, 0:64], cp[:, :, 64:128], ALU.min)
                    if bt == cs + cn - 1:
                        cells_done.add((ch, cs))
                        wsl = slice(cs - 4, cs - 4 + cn)
                        _chain(nc, trp, l1c[(ch, cs)],
                               statw[:, wsl, ksl, :], [cn, 8], 64, W,
                               ALU.max, f"x{ch}{cs}")
                        if not fast:
                            _chain(nc, trp, l1m[(ch, cs)],
                                   minw[:, wsl, ksl, :], [cn, 8], 64, W,
                                   ALU.min, f"n{ch}{cs}")
                        if all((hh, ss) in cells_done
                               for (hh, ss, _) in CELLS if hh == ch):
                            # whole h-half of statw finished -> stream out
                            nc.sync.dma_start(
                                statwd[:, :, ksl, :], statw[:, :, ksl, :])
                            if not fast:
                                nc.scalar.dma_start(
                                    minwd[:, :, ksl, :], minw[:, :, ksl, :])

            nc.sync.dma_start(statd[:, :, :], stat[:, :, :])
            nc.sync.dma_start(mind[:, :] if fast else mind[:, :, :],
                              minstat[:, :] if fast else minstat[:, :, :])

    nc.compile()
    _PROG_CACHE[fast] = nc
    return nc


def _quant(x):
    return np.clip(x * SCALE, -240.0, 240.0).astype(ml_dtypes.float8_e4m3fn)


def kernel(feats, feats_s, labels, labels_s, topk, num_instances):
    global LAST_RESULT
    feats = np.asarray(feats, dtype=np.float32)
    feats_s = np.asarray(feats_s, dtype=np.float32)
    labels = np.asarray(labels).astype(np.int64).ravel()
    labels_s = np.asarray(labels_s).astype(np.int64).ravel()
    tk, ni = int(topk), int(num_instances)
    assert feats.shape == (B, C), feats.shape
    assert tk * ni == G and feats_s.shape == (B, tk, C)

    Fs = feats_s.reshape(-1, C)                       # [16384, 256]
    glab = labels_s.reshape(KTOT, G)[:, 0]            # label of each block

    fast = bool(np.array_equal(labels_s, np.repeat(labels, tk)))
    if fast:
        for j in range(NCORES):
            own = np.isin(labels, glab[j * KLOC:(j + 1) * KLOC])
            want = np.zeros(B, dtype=bool)
            want[j * (B // NCORES):(j + 1) * (B // NCORES)] = True
            if not np.array_equal(own, want):
                fast = False
                break

    nc = _build(fast)

    in_maps = []
    for j in range(NCORES):
        shift = (B // NCORES) * j
        f_loc = np.roll(feats, -shift, axis=0) if fast else feats
        ftT = np.ascontiguousarray(
            f_loc.T.reshape(2, 128, B).transpose(1, 0, 2))
        fsT = Fs[j * MLOC:(j + 1) * MLOC].T.reshape(2, 128, MLOC)
        fsT = np.ascontiguousarray(fsT.transpose(1, 0, 2))   # [kp, kt, n]
        in_maps.append({
            "ftq0": _quant(ftT[:, :, 0:1024]),
            "ftq1": _quant(ftT[:, :, 1024:2048]),
            "fsq0": _quant(fsT[:, :, 0:1024]),
            "fsq1": _quant(fsT[:, :, 1024:2048]),
        })

    LAST_RESULT = run_bass_kernel_spmd(nc, in_maps, core_ids=list(range(NCORES)))

    # ---- host gather/unshard: exp, masks, cross-core sum, -log mean ----
    inv = 1.0 / (TEMP * SCALE * SCALE)
    pos = np.zeros(B, dtype=np.float64)
    neg = np.zeros(B, dtype=np.float64)
    for j in range(NCORES):
        res = LAST_RESULT.results[j]
        gl_j = glab[j * KLOC:(j + 1) * KLOC]              # [16]
        s03 = np.asarray(res["stat"], dtype=np.float32)   # [128, 4, 16]
        sw = np.asarray(res["statw"], dtype=np.float32).max(axis=-1)
        s = np.concatenate([s03, sw], axis=1)             # [128, 16, 16]
        emax = np.exp(s.transpose(1, 0, 2).reshape(B, KLOC) * inv)
        lab_loc = np.roll(labels, -(B // NCORES) * j) if fast else labels
        gmask = lab_loc[:, None] == gl_j[None, :]         # [2048, 16]
        negj = np.where(gmask, 0.0, emax).sum(axis=1)
        mn = np.asarray(res["mins"], dtype=np.float32)
        posj = np.zeros(B, dtype=np.float64)
        if fast:
            emin = np.exp(mn * inv)                       # [128, 2, 8]
            for t in range(2):
                rows = slice(t * 128, (t + 1) * 128)
                gm = gmask[rows, t * 8:(t + 1) * 8]       # [128, 8]
                posj[rows] = np.where(gm, emin[:, t, :], 0.0).sum(axis=1)
        else:
            mw = np.asarray(res["minw"], dtype=np.float32).min(axis=-1)
            m_all = np.concatenate([mn, mw], axis=1)      # [128, 16, 16]
            emin = np.exp(m_all.transpose(1, 0, 2).reshape(B, KLOC) * inv)
            posj = np.where(gmask, emin, 0.0).sum(axis=1)
        if fast:
            shift = (B // NCORES) * j
            negj = np.roll(negj, shift)
            posj = np.roll(posj, shift)
        pos += posj
        neg += negj
    loss_i = -np.log(pos / (pos + neg + EPS) + EPS)
    return np.float32(loss_i.mean())


# revision 11
# speedup vs baseline: 1.3069x; 1.0157x over previous
"""Trainium2 Bass kernel for nn_CriterionLP (LP contrastive criterion loss).

Reference computation (B=2048 anchors, M=16384 supports, C=256, K=128 label
groups of G=128 supports each):
    sim   = (feats @ Fs.T) / TEMP                  [B, M]
    E     = exp(sim) grouped into K blocks of G    [B, K, G]
    pos   = exp(min sim over own-label block)      (one block per row)
    neg   = sum over other blocks of exp(max sim over block)
    loss  = mean_b( -log(pos/(pos+neg+eps) + eps) )

v16 design (support-sharded, 8 cores, no on-device collective):
  - fp8 e4m3 DoubleRow matmuls: full C=256 contraction per instruction,
    ~250ns effective per [128, 512] output; half the input DMA of bf16.
    fp8 end-to-end rel err ~6e-4 against a 2e-2 tolerance.
  - work unit = half b-tile [128, 1024] PSUM (2 banks, 4-deep ring) so
    the PE is never gated by a slow consumer holding a big tile.
  - TRN2 engine rules: gpsimd has no PSUM access and no max; Act has no
    max; DVE reads at most ONE PSUM operand per op. Split:
      D-units (8, b-tiles 0-3): one DVE tensor_reduce straight off PSUM
              -> final [128, 8] block maxes (W=1).
      A2-units (24, b-tiles 4-15): Act copies the two 64-col halves of
              each block separately to bf16 SBUF; DVE TT-maxes them at
              2x, then short 2x chains batched per (h, bt-range) cell
              down to W=16; the HOST finishes max-of-16.
    This balances Act (~29us) and DVE (~29us); PE (~16us) hides under.
  - min stats are needed only for own-label blocks; after the row
    rotation they live in D-units (0,h0) and (1,h1): two extra DVE
    min-reduces off the same PSUM.
  - outputs: stat [128,4,16] (W=1 rows 0-3), statw [128,12,16,16] (W=16
    rows 4-15, DMA'd in two h-chunks as cells finish), mins [128,2,8].
    The HOST does exp, label masks, cross-core sum, -log mean (the
    gather/unshard step). No AllReduce, no cross-core barrier.
"""

import numpy as np
import ml_dtypes

import concourse.bass as bass
import concourse.bacc as bacc
import concourse.tile as tile
import concourse.mybir as mybir
from concourse.bass_utils import run_bass_kernel_spmd

VERSION_TAG = "v18"

F32 = mybir.dt.float32
BF16 = mybir.dt.bfloat16
F8 = mybir.dt.float8e4
AX = mybir.AxisListType
ALU = mybir.AluOpType
DRMODE = mybir.MatmulPerfMode.DoubleRow

TEMP = 0.05
EPS = 1e-6
SCALE = 16.0                # fp8 quantization scale (scores come out *S^2)
B, C = 2048, 256
NCORES = 8
KTOT, G = 128, 128          # label groups, supports per group
MLOC = 2048                 # support rows per core
KLOC = KTOT // NCORES       # groups per core (16)
NBT = B // 128              # b tiles of 128 rows (16)
W = 16                      # shipped stat width for A2 rows
NA2B = NBT - 4              # A2 b-tiles (4..15)

# A2 chain cells: (h, first bt, n bts). 8-cells amortize chain overhead.
CELLS = [(0, 4, 8), (0, 12, 4), (1, 4, 8), (1, 12, 4)]

_PROG_CACHE = {}
LAST_RESULT = None          # BassKernelResults of the most recent run


def _units():
    d = [(0, 0), (1, 1), (0, 1), (1, 0), (2, 0), (2, 1), (3, 0), (3, 1)]
    a2 = [(bt, 0) for bt in range(4, 16)] + [(bt, 1) for bt in range(4, 16)]
    order = []
    di, ai = 0, 0
    for i in range(32):
        if i % 4 == 0 and di < len(d):
            order.append(("D", d[di])); di += 1
        else:
            order.append(("A", a2[ai])); ai += 1
    while ai < len(a2):
        order.append(("A", a2[ai])); ai += 1
    return order


def _chain(nc, pool, src, out_ap, dims, w0, w1, op, tag):
    """[128, *dims, w0] bf16 -> [128, *dims, w1] via TT halving (2x)."""
    cur, w = src, w0
    while w > w1:
        hw = w // 2
        if hw == w1:
            nxt_ap = out_ap
        else:
            nxt = pool.tile([128, *dims, hw], BF16, name=f"c{hw}{tag}",
                            tag=f"c{hw}_{'_'.join(map(str, dims))}", bufs=2)
            nxt_ap = nxt[:]
        sel0 = (slice(None),) * (1 + len(dims)) + (slice(0, hw),)
        sel1 = (slice(None),) * (1 + len(dims)) + (slice(hw, w),)
        nc.vector.tensor_tensor(nxt_ap, cur[sel0], cur[sel1], op)
        if hw > w1:
            cur = nxt
        w = hw


def _build(fast):
    if fast in _PROG_CACHE:
        return _PROG_CACHE[fast]

    nc = bacc.Bacc("TRN2", target_bir_lowering=False, debug=False,
                   num_devices=NCORES)
    ft0d = nc.dram_tensor("ftq0", [128, 2, 1024], F8, kind="ExternalInput")
    ft1d = nc.dram_tensor("ftq1", [128, 2, 1024], F8, kind="ExternalInput")
    fs0d = nc.dram_tensor("fsq0", [128, 2, 1024], F8, kind="ExternalInput")
    fs1d = nc.dram_tensor("fsq1", [128, 2, 1024], F8, kind="ExternalInput")
    statd = nc.dram_tensor("stat", [128, 4, KLOC], BF16,
                           kind="ExternalOutput")
    statwd = nc.dram_tensor("statw", [128, 2, NA2B, 8, W], BF16,
                            kind="ExternalOutput")
    mind = nc.dram_tensor("mins", [128, 2, 8] if fast else [128, 4, KLOC],
                          BF16, kind="ExternalOutput")
    if not fast:
        minwd = nc.dram_tensor("minw", [128, 2, NA2B, 8, W], BF16,
                               kind="ExternalOutput")

    units = _units()

    with tile.TileContext(nc) as tc:
        with (
            tc.tile_pool(name="wpool", bufs=1) as wp,
            tc.tile_pool(name="cpool", bufs=4) as cpp,
            tc.tile_pool(name="bpool", bufs=1) as bp,
            tc.tile_pool(name="tpool", bufs=2) as trp,
            tc.tile_pool(name="pspool", bufs=4, space="PSUM") as psp,
        ):
            ftt = [wp.tile([128, 2, 1024], F8, name=f"ft{c}") for c in range(2)]
            fst = [wp.tile([128, 2, 1024], F8, name=f"fs{c}") for c in range(2)]
            nc.sync.dma_start(ftt[0][:, :, :], ft0d[:, :, :])
            nc.scalar.dma_start(fst[0][:, :, :], fs0d[:, :, :])
            nc.sync.dma_start(fst[1][:, :, :], fs1d[:, :, :])
            nc.sync.dma_start(ftt[1][:, :, :], ft1d[:, :, :])

            stat = bp.tile([128, 4, KLOC], BF16, name="stat")
            statw = bp.tile([128, 2, NA2B, 8, W], BF16, name="statw")
            minstat = bp.tile([128, 2, 8] if fast else [128, 4, KLOC],
                              BF16, name="minstat")
            if not fast:
                minw = bp.tile([128, 2, NA2B, 8, W], BF16, name="minw")

            # L1-output collection tiles per cell
            l1c = {(h, s): bp.tile([128, n, 8, 64], BF16, name=f"l1c{h}{s}")
                   for (h, s, n) in CELLS}
            if not fast:
                l1m = {(h, s): bp.tile([128, n, 8, 64], BF16,
                                       name=f"l1m{h}{s}")
                       for (h, s, n) in CELLS}
            cells_done = set()

            for kind, (bt, h) in units:
                ps = psp.tile([128, 1024], F32, name=f"ps{bt}_{h}", tag="ps")
                ftc = ftt[bt // 8]
                bl = (bt % 8) * 128
                for m in range(2):
                    nc.tensor.matmul(
                        ps[:, m * 512:(m + 1) * 512],
                        ftc[:, :, bl:bl + 128],
                        fst[h][:, :, m * 512:(m + 1) * 512],
                        start=True, stop=True, perf_mode=DRMODE,
                    )
                ps3 = ps.rearrange("p (k g) -> p k g", g=128)
                ksl = slice(h * 8, (h + 1) * 8)
                if kind == "D":
                    nc.vector.tensor_reduce(stat[:, bt, ksl], ps3[:],
                                            axis=AX.X, op=ALU.max)
                    if fast:
                        if (bt, h) in ((0, 0), (1, 1)):
                            nc.vector.tensor_reduce(minstat[:, bt, :],
                                                    ps3[:], axis=AX.X,
                                                    op=ALU.min)
                    else:
                        nc.vector.tensor_reduce(minstat[:, bt, ksl], ps3[:],
                                                axis=AX.X, op=ALU.min)
                else:
                    cp = cpp.tile([128, 8, 128], BF16, name=f"cp{bt}_{h}",
                                  tag="cp")
                    nc.scalar.copy(cp[:, :, :], ps[:, :])
                    # cell membership
                    for (ch, cs, cn) in CELLS:
                        if ch == h and cs <= bt < cs + cn:
                            break
                    nc.vector.tensor_tensor(l1c[(ch, cs)][:, bt - cs, :, :],
                                            cp[:, :, 0:64], cp[:, :, 64:128],
                                            ALU.max)
                    if not fast:
                        nc.vector.tensor_tensor(
                            l1m[(ch, cs)][:, bt - cs, :, :],
                            cp[:, :, 0:64], cp[:, :, 64:128], ALU.min)
                    if bt == cs + cn - 1:
                        cells_done.add((ch, cs))
                        wsl = slice(cs - 4, cs - 4 + cn)
                        _chain(nc, trp, l1c[(ch, cs)],
                               statw[:, ch, wsl, :, :], [cn, 8], 64, W,
                               ALU.max, f"x{ch}{cs}")
                        if not fast:
                            _chain(nc, trp, l1m[(ch, cs)],
                                   minw[:, ch, wsl, :, :], [cn, 8], 64, W,
                                   ALU.min, f"n{ch}{cs}")
                        if all((hh, ss) in cells_done
                               for (hh, ss, _) in CELLS if hh == ch):
                            # whole h-half of statw finished -> stream out
                            # (h-major layout: contiguous 3KB per partition)
                            nc.sync.dma_start(
                                statwd[:, ch, :, :, :], statw[:, ch, :, :, :])
                            if not fast:
                                nc.sync.dma_start(
                                    minwd[:, ch, :, :, :], minw[:, ch, :, :, :])

            nc.sync.dma_start(statd[:, :, :], stat[:, :, :])
            nc.sync.dma_start(mind[:, :] if fast else mind[:, :, :],
                              minstat[:, :] if fast else minstat[:, :, :])

    nc.compile()
    _PROG_CACHE[fast] = nc
    return nc


def _quant(x):
    return np.clip(x * SCALE, -240.0, 240.0).astype(ml_dtypes.float8_e4m3fn)


def kernel(feats, feats_s, labels, labels_s, topk, num_instances):
    global LAST_RESULT
    feats = np.asarray(feats, dtype=np.float32)
    feats_s = np.asarray(feats_s, dtype=np.float32)
    labels = np.asarray(labels).astype(np.int64).ravel()
    labels_s = np.asarray(labels_s).astype(np.int64).ravel()
    tk, ni = int(topk), int(num_instances)
    assert feats.shape == (B, C), feats.shape
    assert tk * ni == G and feats_s.shape == (B, tk, C)

    Fs = feats_s.reshape(-1, C)                       # [16384, 256]
    glab = labels_s.reshape(KTOT, G)[:, 0]            # label of each block

    fast = bool(np.array_equal(labels_s, np.repeat(labels, tk)))
    if fast:
        for j in range(NCORES):
            own = np.isin(labels, glab[j * KLOC:(j + 1) * KLOC])
            want = np.zeros(B, dtype=bool)
            want[j * (B // NCORES):(j + 1) * (B // NCORES)] = True
            if not np.array_equal(own, want):
                fast = False
                break

    nc = _build(fast)

    in_maps = []
    for j in range(NCORES):
        shift = (B // NCORES) * j
        f_loc = np.roll(feats, -shift, axis=0) if fast else feats
        ftT = np.ascontiguousarray(
            f_loc.T.reshape(2, 128, B).transpose(1, 0, 2))
        fsT = Fs[j * MLOC:(j + 1) * MLOC].T.reshape(2, 128, MLOC)
        fsT = np.ascontiguousarray(fsT.transpose(1, 0, 2))   # [kp, kt, n]
        in_maps.append({
            "ftq0": _quant(ftT[:, :, 0:1024]),
            "ftq1": _quant(ftT[:, :, 1024:2048]),
            "fsq0": _quant(fsT[:, :, 0:1024]),
            "fsq1": _quant(fsT[:, :, 1024:2048]),
        })

    LAST_RESULT = run_bass_kernel_spmd(nc, in_maps, core_ids=list(range(NCORES)))

    # ---- host gather/unshard: exp, masks, cross-core sum, -log mean ----
    inv = 1.0 / (TEMP * SCALE * SCALE)
    pos = np.zeros(B, dtype=np.float64)
    neg = np.zeros(B, dtype=np.float64)
    for j in range(NCORES):
        res = LAST_RESULT.results[j]
        gl_j = glab[j * KLOC:(j + 1) * KLOC]              # [16]
        s03 = np.asarray(res["stat"], dtype=np.float32)   # [128, 4, 16]
        # statw [p, h, i, g, w] -> max over w -> [p, bt, h*8+g]
        sw = np.asarray(res["statw"], dtype=np.float32).max(axis=-1)
        sw = sw.transpose(0, 2, 1, 3).reshape(128, NA2B, KLOC)
        s = np.concatenate([s03, sw], axis=1)             # [128, 16, 16]
        emax = np.exp(s.transpose(1, 0, 2).reshape(B, KLOC) * inv)
        lab_loc = np.roll(labels, -(B // NCORES) * j) if fast else labels
        gmask = lab_loc[:, None] == gl_j[None, :]         # [2048, 16]
        negj = np.where(gmask, 0.0, emax).sum(axis=1)
        mn = np.asarray(res["mins"], dtype=np.float32)
        posj = np.zeros(B, dtype=np.float64)
        if fast:
            emin = np.exp(mn * inv)                       # [128, 2, 8]
            for t in range(2):
                rows = slice(t * 128, (t + 1) * 128)
                gm = gmask[rows, t * 8:(t + 1) * 8]       # [128, 8]
                posj[rows] = np.where(gm, emin[:, t, :], 0.0).sum(axis=1)
        else:
            mw = np.asarray(res["minw"], dtype=np.float32).min(axis=-1)
            mw = mw.transpose(0, 2, 1, 3).reshape(128, NA2B, KLOC)
            m_all = np.concatenate([mn, mw], axis=1)      # [128, 16, 16]
            emin = np.exp(m_all.transpose(1, 0, 2).reshape(B, KLOC) * inv)
            posj = np.where(gmask, emin, 0.0).sum(axis=1)
        if fast:
            shift = (B // NCORES) * j
            negj = np.roll(negj, shift)
            posj = np.roll(posj, shift)
        pos += posj
        neg += negj
    loss_i = -np.log(pos / (pos + neg + EPS) + EPS)
    return np.float32(loss_i.mean())


# revision 13
# speedup vs baseline: 1.3082x; 1.0010x over previous
"""Trainium2 Bass kernel for nn_CriterionLP (LP contrastive criterion loss).

Reference computation (B=2048 anchors, M=16384 supports, C=256, K=128 label
groups of G=128 supports each):
    sim   = (feats @ Fs.T) / TEMP                  [B, M]
    E     = exp(sim) grouped into K blocks of G    [B, K, G]
    pos   = exp(min sim over own-label block)      (one block per row)
    neg   = sum over other blocks of exp(max sim over block)
    loss  = mean_b( -log(pos/(pos+neg+eps) + eps) )

v16 design (support-sharded, 8 cores, no on-device collective):
  - fp8 e4m3 DoubleRow matmuls: full C=256 contraction per instruction,
    ~250ns effective per [128, 512] output; half the input DMA of bf16.
    fp8 end-to-end rel err ~6e-4 against a 2e-2 tolerance.
  - work unit = half b-tile [128, 1024] PSUM (2 banks, 4-deep ring) so
    the PE is never gated by a slow consumer holding a big tile.
  - TRN2 engine rules: gpsimd has no PSUM access and no max; Act has no
    max; DVE reads at most ONE PSUM operand per op. Split:
      D-units (8, b-tiles 0-3): one DVE tensor_reduce straight off PSUM
              -> final [128, 8] block maxes (W=1).
      A2-units (24, b-tiles 4-15): Act copies the two 64-col halves of
              each block separately to bf16 SBUF; DVE TT-maxes them at
              2x, then short 2x chains batched per (h, bt-range) cell
              down to W=16; the HOST finishes max-of-16.
    This balances Act (~29us) and DVE (~29us); PE (~16us) hides under.
  - min stats are needed only for own-label blocks; after the row
    rotation they live in D-units (0,h0) and (1,h1): two extra DVE
    min-reduces off the same PSUM.
  - outputs: stat [128,4,16] (W=1 rows 0-3), statw [128,12,16,16] (W=16
    rows 4-15, DMA'd in two h-chunks as cells finish), mins [128,2,8].
    The HOST does exp, label masks, cross-core sum, -log mean (the
    gather/unshard step). No AllReduce, no cross-core barrier.
"""

import numpy as np
import ml_dtypes

import concourse.bass as bass
import concourse.bacc as bacc
import concourse.tile as tile
import concourse.mybir as mybir
from concourse.bass_utils import run_bass_kernel_spmd

VERSION_TAG = "v18"

F32 = mybir.dt.float32
BF16 = mybir.dt.bfloat16
F8 = mybir.dt.float8e4
AX = mybir.AxisListType
ALU = mybir.AluOpType
DRMODE = mybir.MatmulPerfMode.DoubleRow

TEMP = 0.05
EPS = 1e-6
SCALE = 16.0                # fp8 quantization scale (scores come out *S^2)
B, C = 2048, 256
NCORES = 8
KTOT, G = 128, 128          # label groups, supports per group
MLOC = 2048                 # support rows per core
KLOC = KTOT // NCORES       # groups per core (16)
NBT = B // 128              # b tiles of 128 rows (16)
W = 16                      # shipped stat width for A2 rows
NA2B = NBT - 5              # A2 b-tiles (5..15)

# A2 chain cells: (h, first bt, n bts). Wide cells amortize chain overhead.
CELLS = [(0, 5, 7), (0, 12, 4), (1, 5, 7), (1, 12, 4)]

_PROG_CACHE = {}
LAST_RESULT = None          # BassKernelResults of the most recent run


def _units():
    d = [(0, 0), (1, 1), (0, 1), (1, 0), (2, 0), (2, 1), (3, 0), (3, 1),
         (4, 0), (4, 1)]
    a2 = [(bt, 0) for bt in range(5, 16)] + [(bt, 1) for bt in range(5, 16)]
    order = []
    di, ai = 0, 0
    for i in range(32):
        if i % 3 == 0 and di < len(d):
            order.append(("D", d[di])); di += 1
        elif ai < len(a2):
            order.append(("A", a2[ai])); ai += 1
        else:
            order.append(("D", d[di])); di += 1
    return order


def _chain(nc, pool, src, out_ap, dims, w0, w1, op, tag):
    """[128, *dims, w0] bf16 -> [128, *dims, w1] via TT halving (2x)."""
    cur, w = src, w0
    while w > w1:
        hw = w // 2
        if hw == w1:
            nxt_ap = out_ap
        else:
            nxt = pool.tile([128, *dims, hw], BF16, name=f"c{hw}{tag}",
                            tag=f"c{hw}_{'_'.join(map(str, dims))}", bufs=2)
            nxt_ap = nxt[:]
        sel0 = (slice(None),) * (1 + len(dims)) + (slice(0, hw),)
        sel1 = (slice(None),) * (1 + len(dims)) + (slice(hw, w),)
        nc.vector.tensor_tensor(nxt_ap, cur[sel0], cur[sel1], op)
        if hw > w1:
            cur = nxt
        w = hw


def _build(fast):
    if fast in _PROG_CACHE:
        return _PROG_CACHE[fast]

    nc = bacc.Bacc("TRN2", target_bir_lowering=False, debug=False,
                   num_devices=NCORES)
    ft0d = nc.dram_tensor("ftq0", [128, 2, 1024], F8, kind="ExternalInput")
    ft1d = nc.dram_tensor("ftq1", [128, 2, 1024], F8, kind="ExternalInput")
    fs0d = nc.dram_tensor("fsq0", [128, 2, 1024], F8, kind="ExternalInput")
    fs1d = nc.dram_tensor("fsq1", [128, 2, 1024], F8, kind="ExternalInput")
    statd = nc.dram_tensor("stat", [128, 5, KLOC], BF16,
                           kind="ExternalOutput")
    statwd = nc.dram_tensor("statw", [128, 2, NA2B, 8, W], BF16,
                            kind="ExternalOutput")
    mind = nc.dram_tensor("mins", [128, 2, 8] if fast else [128, 5, KLOC],
                          BF16, kind="ExternalOutput")
    if not fast:
        minwd = nc.dram_tensor("minw", [128, 2, NA2B, 8, W], BF16,
                               kind="ExternalOutput")

    units = _units()

    with tile.TileContext(nc) as tc:
        with (
            tc.tile_pool(name="wpool", bufs=1) as wp,
            tc.tile_pool(name="cpool", bufs=6) as cpp,
            tc.tile_pool(name="bpool", bufs=1) as bp,
            tc.tile_pool(name="tpool", bufs=2) as trp,
            tc.tile_pool(name="pspool", bufs=4, space="PSUM") as psp,
        ):
            ftt = [wp.tile([128, 2, 1024], F8, name=f"ft{c}") for c in range(2)]
            fst = [wp.tile([128, 2, 1024], F8, name=f"fs{c}") for c in range(2)]
            nc.sync.dma_start(ftt[0][:, :, :], ft0d[:, :, :])
            nc.scalar.dma_start(fst[0][:, :, :], fs0d[:, :, :])
            nc.sync.dma_start(fst[1][:, :, :], fs1d[:, :, :])
            nc.sync.dma_start(ftt[1][:, :, :], ft1d[:, :, :])

            stat = bp.tile([128, 5, KLOC], BF16, name="stat")
            statw = bp.tile([128, 2, NA2B, 8, W], BF16, name="statw")
            minstat = bp.tile([128, 2, 8] if fast else [128, 5, KLOC],
                              BF16, name="minstat")
            if not fast:
                minw = bp.tile([128, 2, NA2B, 8, W], BF16, name="minw")

            # L1-output collection tiles per cell
            l1c = {(h, s): bp.tile([128, n, 8, 64], BF16, name=f"l1c{h}{s}")
                   for (h, s, n) in CELLS}
            if not fast:
                l1m = {(h, s): bp.tile([128, n, 8, 64], BF16,
                                       name=f"l1m{h}{s}")
                       for (h, s, n) in CELLS}
            cells_done = set()

            for kind, (bt, h) in units:
                ps = psp.tile([128, 1024], F32, name=f"ps{bt}_{h}", tag="ps")
                ftc = ftt[bt // 8]
                bl = (bt % 8) * 128
                for m in range(2):
                    nc.tensor.matmul(
                        ps[:, m * 512:(m + 1) * 512],
                        ftc[:, :, bl:bl + 128],
                        fst[h][:, :, m * 512:(m + 1) * 512],
                        start=True, stop=True, perf_mode=DRMODE,
                    )
                ps3 = ps.rearrange("p (k g) -> p k g", g=128)
                ksl = slice(h * 8, (h + 1) * 8)
                if kind == "D":
                    nc.vector.tensor_reduce(stat[:, bt, ksl], ps3[:],
                                            axis=AX.X, op=ALU.max)
                    if fast:
                        if (bt, h) in ((0, 0), (1, 1)):
                            nc.vector.tensor_reduce(minstat[:, bt, :],
                                                    ps3[:], axis=AX.X,
                                                    op=ALU.min)
                    else:
                        nc.vector.tensor_reduce(minstat[:, bt, ksl], ps3[:],
                                                axis=AX.X, op=ALU.min)
                else:
                    cp = cpp.tile([128, 8, 128], BF16, name=f"cp{bt}_{h}",
                                  tag="cp")
                    nc.scalar.copy(cp[:, :, :], ps[:, :])
                    # cell membership
                    for (ch, cs, cn) in CELLS:
                        if ch == h and cs <= bt < cs + cn:
                            break
                    nc.vector.tensor_tensor(l1c[(ch, cs)][:, bt - cs, :, :],
                                            cp[:, :, 0:64], cp[:, :, 64:128],
                                            ALU.max)
                    if not fast:
                        nc.vector.tensor_tensor(
                            l1m[(ch, cs)][:, bt - cs, :, :],
                            cp[:, :, 0:64], cp[:, :, 64:128], ALU.min)
                    if bt == cs + cn - 1:
                        cells_done.add((ch, cs))
                        wsl = slice(cs - 5, cs - 5 + cn)
                        _chain(nc, trp, l1c[(ch, cs)],
                               statw[:, ch, wsl, :, :], [cn, 8], 64, W,
                               ALU.max, f"x{ch}{cs}")
                        if not fast:
                            _chain(nc, trp, l1m[(ch, cs)],
                                   minw[:, ch, wsl, :, :], [cn, 8], 64, W,
                                   ALU.min, f"n{ch}{cs}")
                        if all((hh, ss) in cells_done
                               for (hh, ss, _) in CELLS if hh == ch):
                            # whole h-half of statw finished -> stream out
                            # (h-major layout: contiguous 3KB per partition)
                            nc.sync.dma_start(
                                statwd[:, ch, :, :, :], statw[:, ch, :, :, :])
                            if not fast:
                                nc.sync.dma_start(
                                    minwd[:, ch, :, :, :], minw[:, ch, :, :, :])

            nc.sync.dma_start(statd[:, :, :], stat[:, :, :])
            nc.sync.dma_start(mind[:, :] if fast else mind[:, :, :],
                              minstat[:, :] if fast else minstat[:, :, :])

    nc.compile()
    _PROG_CACHE[fast] = nc
    return nc


def _quant(x):
    return np.clip(x * SCALE, -240.0, 240.0).astype(ml_dtypes.float8_e4m3fn)


def kernel(feats, feats_s, labels, labels_s, topk, num_instances):
    global LAST_RESULT
    feats = np.asarray(feats, dtype=np.float32)
    feats_s = np.asarray(feats_s, dtype=np.float32)
    labels = np.asarray(labels).astype(np.int64).ravel()
    labels_s = np.asarray(labels_s).astype(np.int64).ravel()
    tk, ni = int(topk), int(num_instances)
    assert feats.shape == (B, C), feats.shape
    assert tk * ni == G and feats_s.shape == (B, tk, C)

    Fs = feats_s.reshape(-1, C)                       # [16384, 256]
    glab = labels_s.reshape(KTOT, G)[:, 0]            # label of each block

    fast = bool(np.array_equal(labels_s, np.repeat(labels, tk)))
    if fast:
        for j in range(NCORES):
            own = np.isin(labels, glab[j * KLOC:(j + 1) * KLOC])
            want = np.zeros(B, dtype=bool)
            want[j * (B // NCORES):(j + 1) * (B // NCORES)] = True
            if not np.array_equal(own, want):
                fast = False
                break

    nc = _build(fast)

    in_maps = []
    for j in range(NCORES):
        shift = (B // NCORES) * j
        f_loc = np.roll(feats, -shift, axis=0) if fast else feats
        ftT = np.ascontiguousarray(
            f_loc.T.reshape(2, 128, B).transpose(1, 0, 2))
        fsT = Fs[j * MLOC:(j + 1) * MLOC].T.reshape(2, 128, MLOC)
        fsT = np.ascontiguousarray(fsT.transpose(1, 0, 2))   # [kp, kt, n]
        in_maps.append({
            "ftq0": _quant(ftT[:, :, 0:1024]),
            "ftq1": _quant(ftT[:, :, 1024:2048]),
            "fsq0": _quant(fsT[:, :, 0:1024]),
            "fsq1": _quant(fsT[:, :, 1024:2048]),
        })

    LAST_RESULT = run_bass_kernel_spmd(nc, in_maps, core_ids=list(range(NCORES)))

    # ---- host gather/unshard: exp, masks, cross-core sum, -log mean ----
    inv = 1.0 / (TEMP * SCALE * SCALE)
    pos = np.zeros(B, dtype=np.float64)
    neg = np.zeros(B, dtype=np.float64)
    for j in range(NCORES):
        res = LAST_RESULT.results[j]
        gl_j = glab[j * KLOC:(j + 1) * KLOC]              # [16]
        s03 = np.asarray(res["stat"], dtype=np.float32)   # [128, 5, 16]
        # statw [p, h, i, g, w] -> max over w -> [p, bt, h*8+g]
        sw = np.asarray(res["statw"], dtype=np.float32).max(axis=-1)
        sw = sw.transpose(0, 2, 1, 3).reshape(128, NA2B, KLOC)
        s = np.concatenate([s03, sw], axis=1)             # [128, 16, 16]
        emax = np.exp(s.transpose(1, 0, 2).reshape(B, KLOC) * inv)
        lab_loc = np.roll(labels, -(B // NCORES) * j) if fast else labels
        gmask = lab_loc[:, None] == gl_j[None, :]         # [2048, 16]
        negj = np.where(gmask, 0.0, emax).sum(axis=1)
        mn = np.asarray(res["mins"], dtype=np.float32)
        posj = np.zeros(B, dtype=np.float64)
        if fast:
            emin = np.exp(mn * inv)                       # [128, 2, 8]
            for t in range(2):
                rows = slice(t * 128, (t + 1) * 128)
                gm = gmask[rows, t * 8:(t + 1) * 8]       # [128, 8]
                posj[rows] = np.where(gm, emin[:, t, :], 0.0).sum(axis=1)
        else:
            mw = np.asarray(res["minw"], dtype=np.float32).min(axis=-1)
            mw = mw.transpose(0, 2, 1, 3).reshape(128, NA2B, KLOC)
            m_all = np.concatenate([mn, mw], axis=1)      # [128, 16, 16]
            emin = np.exp(m_all.transpose(1, 0, 2).reshape(B, KLOC) * inv)
            posj = np.where(gmask, emin, 0.0).sum(axis=1)
        if fast:
            shift = (B // NCORES) * j
            negj = np.roll(negj, shift)
            posj = np.roll(posj, shift)
        pos += posj
        neg += negj
    loss_i = -np.log(pos / (pos + neg + EPS) + EPS)
    return np.float32(loss_i.mean())
